# revision 1
# baseline (speedup 1.0000x reference)
"""Bass/Tile Trainium2 kernel for nn_BilinearAttentionFusion.

Self-contained: fast zero-bias path (fp8 DoubleRow everywhere, batched
activations, DVE/GPSIMD softmax pipeline) plus the original general
builder as a fallback for nonzero biases.  See build_fast's docstring
for the fast-path design notes.
"""

import os
import time

import numpy as np
import ml_dtypes

import concourse.bass as bass
import concourse.tile as tile
from concourse import bacc
from concourse import mybir
from concourse.bass_utils import run_bass_kernel_spmd

BF16 = ml_dtypes.bfloat16

B, S, L, H, C = 16, 2048, 256, 768, 512
NCORES = 8
B_LOC = B // NCORES          # 2 samples per core
S_LOC = B_LOC * S            # 4096 rows per core
SC = 256                     # s-columns per chunk (fast path)
NCHUNK = S_LOC // SC         # 16
NGRP = NCHUNK // 2           # 8 two-chunk groups (= exp quads)
GRP_PER_SMP = NGRP // B_LOC  # 4
KH = H // 128                # 6 k-tiles over H
MC = C // 128                # 4 m-tiles over C

FP32 = mybir.dt.float32
BF = mybir.dt.bfloat16
F8 = mybir.dt.float8e4
AX = mybir.AxisListType
AF = mybir.ActivationFunctionType
ALU = mybir.AluOpType
DR = mybir.MatmulPerfMode.DoubleRow

# general (nonzero-bias) path constants
SC_G = 512
NCHUNK_G = S_LOC // SC_G
NSUB_G = SC_G // 128
CH_PER_SMP_G = S // SC_G

import os as _os
GSB = _os.environ.get("KF_GSB", "both")        # fusion ACT-copy assist: off|s1|both
SCMID = _os.environ.get("KF_SCMID", "2p2")   # mid-quad scale split: 2p2|1p3
E7SPLIT = _os.environ.get("KF_E7", "1") == "1"  # split last exp into halves

_cache = {}

def build_fast():
    nc = bacc.Bacc()

    xT_d = nc.dram_tensor("xT", [H, S_LOC], F8, kind="ExternalInput")
    wcombT_d = nc.dram_tensor("wcombT", [H, 2 * C], F8, kind="ExternalInput")
    wlT_d = nc.dram_tensor("wlT", [H, C], F8, kind="ExternalInput")
    wlaT_d = nc.dram_tensor("wlaT", [H, C], F8, kind="ExternalInput")
    labT_d = nc.dram_tensor("labT", [H, L], F8, kind="ExternalInput")
    wpT_d = nc.dram_tensor("wpT", [C, H], FP32, kind="ExternalInput")
    ctx_d = nc.dram_tensor("ctxcol", [C, 1], FP32, kind="ExternalInput")
    outT_d = nc.dram_tensor("outT", [H, B_LOC], FP32, kind="ExternalOutput")

    with tile.TileContext(nc) as tc, \
            tc.tile_pool(name="sg", bufs=1) as sg, \
            tc.tile_pool(name="pb", bufs=1) as pb, \
            tc.tile_pool(name="px", bufs=3) as px:

        # ---- static SBUF ----
        wcomb_sb = sg.tile([128, KH, 2 * C], F8)
        wl_sb = sg.tile([128, KH, C], F8)
        wla_sb = sg.tile([128, KH, C], F8)
        lab_sb = sg.tile([128, KH, L], F8)
        wp_sb = sg.tile([128, MC, H], FP32)
        ctx_sb = sg.tile([128, MC, 1], FP32)
        shift_sb = sg.tile([128, 1], FP32)
        ltT_sb = sg.tile([128, MC, L], BF)       # label_trans^T  [c, l]
        laX_sb = sg.tile([128, MC, L], F8)       # (ctx*label_attn)^T [c, l]
        la_f = sg.tile([128, MC, L], BF)
        # per-m fusion tiles: one shared tile would WAW-serialize the
        # cross-engine accum writes at tile granularity
        fus_m = [sg.tile([128, B_LOC], FP32, name=f"fus{m}")
                 for m in range(MC)]
        z1 = sg.tile([1, 128], BF)               # PE p-state warmup operands
        z8 = sg.tile([1, 8], BF)

        nc.vector.memset(shift_sb, -64.0)
        nc.vector.memset(z1, 0.0)
        nc.vector.memset(z8, 0.0)

        # ---- DMA order: Wia half first so the ia-ch0 chain starts the
        #      ACT stream ASAP; x groups paced between remaining weights.
        xg = [px.tile([128, KH, 2 * SC], F8, tag="xt", name=f"xg{g}")
              for g in range(NGRP)]
        nc.sync.dma_start(out=wcomb_sb[:, :, C:2 * C],
                          in_=wcombT_d[:, C:2 * C]
                          .rearrange("(k p) n -> p k n", p=128))
        nc.sync.dma_start(
            out=xg[0], in_=xT_d[:, 0:2 * SC].rearrange("(k p) s -> p k s", p=128))
        nc.sync.dma_start(out=wcomb_sb[:, :, 0:C],
                          in_=wcombT_d[:, 0:C]
                          .rearrange("(k p) n -> p k n", p=128))
        nc.sync.dma_start(
            out=xg[1], in_=xT_d[:, 2 * SC:4 * SC].rearrange("(k p) s -> p k s", p=128))
        nc.sync.dma_start(
            out=xg[2], in_=xT_d[:, 4 * SC:6 * SC].rearrange("(k p) s -> p k s", p=128))
        nc.sync.dma_start(out=lab_sb, in_=labT_d.rearrange("(k p) n -> p k n", p=128))
        nc.sync.dma_start(out=wl_sb, in_=wlT_d.rearrange("(k p) n -> p k n", p=128))
        nc.sync.dma_start(
            out=xg[3], in_=xT_d[:, 6 * SC:8 * SC].rearrange("(k p) s -> p k s", p=128))
        nc.sync.dma_start(out=ctx_sb, in_=ctx_d.rearrange("(m p) o -> p m o", p=128))
        nc.sync.dma_start(out=wla_sb, in_=wlaT_d.rearrange("(k p) n -> p k n", p=128))
        for g in range(4, NGRP):
            nc.sync.dma_start(
                out=xg[g],
                in_=xT_d[:, 2 * SC * g:2 * SC * (g + 1)]
                    .rearrange("(k p) s -> p k s", p=128))
        nc.sync.dma_start(out=wp_sb, in_=wpT_d.rearrange("(m p) n -> p m n", p=128))

        ia_tiles = []   # per group [128, 2, MC, SC] f8, iaT[c, s]
        it_tiles = []   # per group [128, 4, C] f8, it[s, c]

        # ---- phase 1: projections over 8 groups (16 chunks) ----
        with tc.tile_pool(name="pia", space="PSUM", bufs=1) as pia, \
                tc.tile_pool(name="pit", space="PSUM", bufs=1) as pit:
            for g in range(NGRP):
                xt = xg[g]
                # iaT[c, s]: both chunks of the group into one 4-bank psum
                ia_ps = pia.tile([128, 2, MC, SC], FP32, tag="ia", name="ia_ps")
                if g == 0:
                    # p-state warmup: one zero matmul with no DMA deps
                    # starts the PE ramp clock at ~t=0.
                    nc.tensor.matmul(ia_ps[:, 1, 3, 248:256], z1, z8,
                                     start=True, stop=True,
                                     skip_group_check=True)
                iaT = pb.tile([128, 2, MC, SC], F8, tag="iaT", bufs=NGRP,
                              name=f"iaT{g}")
                ia_tiles.append(iaT)
                for cc in range(2):
                    for kk in range(KH // 2):
                        for m in range(MC):
                            nc.tensor.matmul(
                                ia_ps[:, cc, m, :],
                                wcomb_sb[:, 2 * kk:2 * kk + 2,
                                         C + 128 * m:C + 128 * (m + 1)],
                                xt[:, 2 * kk:2 * kk + 2, SC * cc:SC * (cc + 1)],
                                start=(kk == 0), stop=(kk == KH // 2 - 1),
                                perf_mode=DR, skip_group_check=True)
                nc.scalar.activation(iaT, ia_ps, AF.Sigmoid)

                # it[s, c]: 4 j-subtiles
                it_ps = pit.tile([128, 4, C], FP32, tag="it", name="it_ps")
                for kk in range(KH // 2):
                    for j in range(4):
                        nc.tensor.matmul(
                            it_ps[:, j, :],
                            xt[:, 2 * kk:2 * kk + 2, 128 * j:128 * (j + 1)],
                            wcomb_sb[:, 2 * kk:2 * kk + 2, 0:C],
                            start=(kk == 0), stop=(kk == KH // 2 - 1),
                            perf_mode=DR, skip_group_check=True)
                itN = pb.tile([128, 4, C], F8, tag="itN", bufs=NGRP,
                              name=f"itN{g}")
                it_tiles.append(itN)
                nc.scalar.activation(itN, it_ps, AF.Sigmoid)

            # label branches ride the freed projection psum rings right
            # after group 7 (lt in the ia ring, la in the it ring) so
            # their acts butt against the last projection sigmoids
            for which in range(2):
                w_sb = wl_sb if which == 0 else wla_sb
                if which == 0:
                    ps = pia.tile([128, 2, MC, SC], FP32, tag="ia",
                                  name="lab_ps")[:, 0]
                else:
                    ps = pit.tile([128, 4, C], FP32, tag="it",
                                  name="lab_ps2")
                for m in range(MC):
                    for kk in range(KH // 2):
                        nc.tensor.matmul(
                            ps[:, m, 0:L],
                            w_sb[:, 2 * kk:2 * kk + 2, 128 * m:128 * (m + 1)],
                            lab_sb[:, 2 * kk:2 * kk + 2, :],
                            start=(kk == 0), stop=(kk == KH // 2 - 1),
                            perf_mode=DR, skip_group_check=True)
                if which == 0:
                    nc.scalar.activation(ltT_sb, ps[:, :, 0:L], AF.Sigmoid)
                else:
                    nc.scalar.activation(la_f, ps[:, :, 0:L], AF.Sigmoid)
                    for m in range(MC):
                        nc.vector.tensor_scalar_mul(
                            laX_sb[:, m, :], la_f[:, m, :], ctx_sb[:, m, :])

            # tiny Exp pinned after the last sigmoid (reads la output): the
            # auto-inserted exp-table load binds here and hides the phase-2
            # psum pool-boundary latency instead of delaying the first Exp
            dummy_e = sg.tile([128, 1], FP32)
            nc.scalar.activation(dummy_e, la_f[:, 0, 0:1], AF.Exp)

        # ---- phase 2: logits -> softmax -> G -> fusion as one global
        #      quad pipeline over both samples (n = 4*smp+q) ----
        with tc.tile_pool(name="plg", space="PSUM", bufs=2) as plg:
            G_tiles = {}
            eb_tiles = {}

            def emit_logits(n):
                smp, q = divmod(n, GRP_PER_SMP)
                lg = plg.tile([128, 4, L], FP32, tag="lg", name="lg_ps")
                iaT = ia_tiles[n]
                for j in range(4):
                    cc, jj = divmod(j, 2)
                    for mp in range(2):
                        nc.tensor.matmul(
                            lg[:, j, :],
                            iaT[:, cc, 2 * mp:2 * mp + 2,
                                128 * jj:128 * (jj + 1)],
                            laX_sb[:, 2 * mp:2 * mp + 2, :],
                            start=(mp == 0), stop=(mp == 1),
                            perf_mode=DR, skip_group_check=True)
                # softmax numerator + per-row denominators
                ef = pb.tile([128, 4, L], BF, tag="ef", bufs=int(_os.environ.get("KF_EFB","8")), name="ef")
                den = pb.tile([128, 4], FP32, tag="den", bufs=4, name="den")
                dsc = pb.tile([128, L], BF, tag="dsc", bufs=int(_os.environ.get("KF_DSB","2")), name="dsc")
                rr = pb.tile([128, 4], FP32, tag="rr", bufs=4, name="rr")
                eb = pb.tile([128, 4, L], F8, tag="eb", bufs=int(_os.environ.get("KF_EBB","8")), name="eb")
                eb_tiles[n] = eb
                # scale-engine split: last quad all-DVE in two half-chains
                # (critical tail latency); Pool-heavy mid-quads keep the
                # DVE drained for the tail
                if n == 2 * GRP_PER_SMP - 1:
                    engs = [nc.vector] * 4
                elif SCMID == "2p2" or n in (0, 2 * GRP_PER_SMP - 2):
                    engs = [nc.vector if j % 2 == 0 else nc.gpsimd
                            for j in range(4)]
                else:
                    engs = [nc.vector] + [nc.gpsimd] * 3
                E7N = int(_os.environ.get("KF_E7N", "1"))  # split last N quads
                if E7SPLIT and n >= 2 * GRP_PER_SMP - E7N:
                    # split exp into halves so the softmax chain and the
                    # first G pair start half an exp earlier
                    for h in range(2):
                        sl = slice(2 * h, 2 * h + 2)
                        nc.scalar.activation(ef[:, sl], lg[:, sl], AF.Exp,
                                             bias=shift_sb)
                        for j in (2 * h, 2 * h + 1):
                            nc.vector.tensor_scalar(dsc, ef[:, j, :], 1.0,
                                                    None, op0=ALU.mult,
                                                    op1=ALU.add,
                                                    accum_out=den[:, j:j + 1])
                        nc.vector.reciprocal(rr[:, sl], den[:, sl])
                        for j in (2 * h, 2 * h + 1):
                            nc.vector.tensor_scalar(eb[:, j, :], ef[:, j, :],
                                                    rr[:, j:j + 1], 64.0,
                                                    op0=ALU.mult, op1=ALU.mult)
                else:
                    nc.scalar.activation(ef, lg, AF.Exp, bias=shift_sb)
                    NPD = int(_os.environ.get("KF_PDEN", "0"))
                    for j in range(4):
                        deng = (nc.gpsimd if (n < NPD and j % 2 == 1)
                                else nc.vector)
                        deng.tensor_scalar(dsc, ef[:, j, :], 1.0, None,
                                           op0=ALU.mult, op1=ALU.add,
                                           accum_out=den[:, j:j + 1])
                    nc.vector.reciprocal(rr, den)
                    for j in range(4):
                        engs[j].tensor_scalar(eb[:, j, :], ef[:, j, :],
                                              rr[:, j:j + 1], 64.0,
                                              op0=ALU.mult, op1=ALU.mult)

            def emit_G(n):
                smp, q = divmod(n, GRP_PER_SMP)
                if q == 0:
                    G_tiles[smp] = [
                        plg.tile([128, L], FP32, tag="G", bufs=4,
                                 name=f"G{m}_{smp}")
                        for m in range(MC)
                    ]
                Gs = G_tiles[smp]
                itN = it_tiles[GRP_PER_SMP * smp + q]
                eb = eb_tiles[n]
                last = q == GRP_PER_SMP - 1
                if not last:
                    for p in range(2):
                        first = q == 0 and p == 0
                        for m in range(MC):
                            nc.tensor.matmul(
                                Gs[m],
                                itN[:, 2 * p:2 * p + 2, 128 * m:128 * (m + 1)],
                                eb[:, 2 * p:2 * p + 2, :],
                                start=first, stop=False,
                                perf_mode=DR, skip_group_check=True)
                else:
                    # pair-major: pair 0 starts off the DVE-scaled half
                    # while GPSIMD still scales pair 1
                    for p in range(2):
                        for m in range(MC):
                            nc.tensor.matmul(
                                Gs[m],
                                itN[:, 2 * p:2 * p + 2, 128 * m:128 * (m + 1)],
                                eb[:, 2 * p:2 * p + 2, :],
                                start=False, stop=(p == 1),
                                perf_mode=DR, skip_group_check=True)

            def emit_fusion(smp):
                # fusion[c] = (1/64) * sum_l G[c,l] * ltT[c,l]
                Gs = G_tiles[smp]
                # GPSIMD cannot read PSUM, so fusion runs on DVE.  For
                # the tail sample, the by-then-idle ACT engine stages m2/m3
                # into bf16 SBUF, where the DVE reduce hits its 4x mode
                # (127ns vs 392ns).  Mid-phase (smp 0) ACT has no slack.
                if GSB in ("both", "all4") or (GSB == "s1" and smp == 1):
                    for m in ((0, 1, 2, 3) if GSB == "all4" else (2, 3)):
                        gsb = pb.tile([128, L], BF, tag="gsb", bufs=4,
                                      name=f"gsb{m}")
                        nc.scalar.copy(gsb, Gs[m])
                        Gs[m] = gsb
                for m in range(MC):
                    gt = pb.tile([128, L], BF, tag="gt", bufs=4, name="gt")
                    nc.vector.scalar_tensor_tensor(
                        gt, Gs[m], 1.0 / 64.0,
                        ltT_sb[:, m, :],
                        op0=ALU.mult, op1=ALU.mult,
                        accum_out=fus_m[m][:, smp:smp + 1])

            # global software pipeline; DEPTH controls how far G-matmul
            # emission trails logits emission on the PE stream
            DEPTH = int(_os.environ.get("KF_DEPTH", "2"))
            # G's whose index >= GTAIL are all deferred until after the
            # last logits, so PE never stalls the exp stream on Pool-paced
            # eb tiles late in the phase
            GTAIL = int(_os.environ.get("KF_GTAIL", "0"))
            NQ = 2 * GRP_PER_SMP
            emitted_g = 0
            for n in range(NQ):
                emit_logits(n)
                if n + 1 >= DEPTH + 1 and emitted_g < GTAIL:
                    emit_G(emitted_g)
                    emitted_g += 1
                    if emitted_g - 1 == GRP_PER_SMP - 1:
                        emit_fusion(0)
            while emitted_g < NQ:
                emit_G(emitted_g)
                emitted_g += 1
                if emitted_g - 1 == GRP_PER_SMP - 1:
                    emit_fusion(0)
            emit_fusion(1)

            # ---- final projection, transposed (all-fp32): outT[h, b] ----
            o_ps = plg.tile([128, KH, B_LOC], FP32, tag="G", bufs=4,
                            name="o_ps")
            for h in range(KH):
                for m in range(MC):
                    nc.tensor.matmul(
                        o_ps[:, h, :],
                        wp_sb[:, m, 128 * h:128 * (h + 1)],
                        fus_m[m],
                        start=(m == 0), stop=(m == MC - 1))
            outT_sb = sg.tile([128, KH, B_LOC], FP32)
            nc.scalar.copy(outT_sb, o_ps)
            _dq = _os.environ.get("KF_ODMA", "sync")
            _eng = {"sync": nc.sync, "act": nc.scalar, "dve": nc.vector}[_dq]
            _eng.dma_start(
                out=outT_d.rearrange("(k p) b -> p k b", p=128), in_=outT_sb)

    nc.finalize()
    return nc


def host_prep_fast(inputs):
    """Pure layout prep: cast, transpose, concat. No FLOPs."""
    x = np.asarray(inputs["input_hidden_states"], np.float32)
    lab = np.asarray(inputs["label_hidden_states"], np.float32)
    Wi = np.asarray(inputs["Wi"], np.float32)
    Wia = np.asarray(inputs["Wia"], np.float32)
    Wl = np.asarray(inputs["Wl"], np.float32)
    Wla = np.asarray(inputs["Wla"], np.float32)
    Wp = np.asarray(inputs["Wp"], np.float32)
    F8N = ml_dtypes.float8_e4m3

    x_bf = np.ascontiguousarray(x.reshape(B * S, H).T).astype(F8N)  # [H, B*S]
    wcombT = np.ascontiguousarray(
        np.concatenate([Wi, Wia], axis=0).T).astype(F8N)
    wlT = np.ascontiguousarray(Wl.T).astype(F8N)
    wlaT = np.ascontiguousarray(Wla.T).astype(F8N)
    labT = np.ascontiguousarray(lab.T).astype(F8N)
    wpT = np.ascontiguousarray(Wp.T).astype(np.float32)          # [C, H]
    ctxcol = np.asarray(inputs["context"], np.float32).reshape(C, 1)

    shared = dict(wcombT=wcombT, wlT=wlT, wlaT=wlaT, labT=labT, wpT=wpT,
                  ctxcol=ctxcol)
    in_maps = []
    for k in range(NCORES):
        m = dict(shared)
        m["xT"] = np.ascontiguousarray(x_bf[:, k * S_LOC:(k + 1) * S_LOC])
        in_maps.append(m)
    return in_maps


def collect_fast(results):
    return np.concatenate(
        [results[k]["outT"].T for k in range(NCORES)], axis=0
    ).astype(np.float32)


def _build_general(zero_bi=False):
    nc = bacc.Bacc()

    # ---- DRAM I/O ----
    xT_d = nc.dram_tensor("xT", [H, S_LOC], F8, kind="ExternalInput")
    wcombT_d = nc.dram_tensor("wcombT", [H, 2 * C], F8, kind="ExternalInput")
    wlT_d = nc.dram_tensor("wlT", [H, C], BF, kind="ExternalInput")
    wlaT_d = nc.dram_tensor("wlaT", [H, C], BF, kind="ExternalInput")
    labT_d = nc.dram_tensor("labT", [H, L], BF, kind="ExternalInput")
    wpT_d = nc.dram_tensor("wpT", [C, H], BF, kind="ExternalInput")
    bi_d = nc.dram_tensor("bi_row", [1, C], BF, kind="ExternalInput")
    # bvec columns: 0=bia, 1=bl, 2=bla, 3=context
    bvec_d = nc.dram_tensor("bvec", [C, 4], FP32, kind="ExternalInput")
    out_d = nc.dram_tensor("out", [B_LOC, H], FP32, kind="ExternalOutput")

    with tile.TileContext(nc) as tc, \
            tc.tile_pool(name="singles", bufs=1) as sg:
        # ---- static SBUF tensors ----
        wcomb_sb = sg.tile([128, KH, 2 * C], F8)      # [p, k, 1024]
        wl_sb = sg.tile([128, KH, C], BF)
        wla_sb = sg.tile([128, KH, C], BF)
        lab_sb = sg.tile([128, KH, L], BF)
        wp_sb = sg.tile([128, MC, H], BF)
        bi_sb = sg.tile([1, C], BF)
        bias_sb = sg.tile([128, MC, 4], FP32)
        ones_sb = sg.tile([1, 128], BF)
        shift_sb = sg.tile([128, 1], FP32)            # softmax exp shift
        ltT_sb = sg.tile([128, MC, L], BF)            # label_trans^T  [c, l]
        laX_sb = sg.tile([128, MC, L], BF)            # (ctx*label_attn)^T [c, l]
        fus_f = sg.tile([128, 2 * MC], FP32)          # fusion cols: 2*m + smp
        fus_b = sg.tile([128, 2 * MC], BF)
        out_sb = sg.tile([B_LOC, H], FP32)

        nc.vector.memset(ones_sb, 1.0)
        nc.vector.memset(shift_sb, -64.0)
        # DMA queue order matters: the sync HWDGE ring drains FIFO, and PE's
        # first work (label lt matmuls) needs lab+wl while the projections
        # need wcomb + x chunk 0 as soon as possible. Everything else defers.
        nc.sync.dma_start(out=lab_sb, in_=labT_d.rearrange("(k p) n -> p k n", p=128))
        nc.sync.dma_start(out=wl_sb, in_=wlT_d.rearrange("(k p) n -> p k n", p=128))
        nc.sync.dma_start(out=bias_sb, in_=bvec_d.rearrange("(m p) c -> p m c", p=128))
        nc.sync.dma_start(out=bi_sb, in_=bi_d[:, :])
        nc.sync.dma_start(out=wcomb_sb, in_=wcombT_d.rearrange("(k p) n -> p k n", p=128))

        # ---- phase 1: projections over 8 chunks ----
        ia_tiles = []   # per chunk: [128, MC, SC_G] bf16, iaT[c, s]
        it_tiles = []   # per chunk: [128, NSUB_G, C] bf16, it[s, c]
        with tc.tile_pool(name="pacts", bufs=NCHUNK) as pacts:
            with (tc.tile_pool(name="px", bufs=3) as px,
                  tc.tile_pool(name="pp0", space="PSUM", bufs=2) as pp0,
                  tc.tile_pool(name="ppc", space="PSUM", bufs=6) as ppc):
                # label lt matmuls fill the PE while wcomb + x chunk 0 stream in
                for m in range(MC):
                    lt_ps = pp0.tile([128, L], FP32, tag="lbl")
                    for k in range(KH):
                        nc.tensor.matmul(
                            lt_ps, wl_sb[:, k, 128 * m:128 * (m + 1)],
                            lab_sb[:, k, :],
                            start=(k == 0), stop=(k == KH - 1))
                    nc.scalar.activation(ltT_sb[:, m, :], lt_ps, AF.Sigmoid,
                                         bias=bias_sb[:, m, 1:2])

                for ch in range(NCHUNK_G if True else 0):
                    xt = px.tile([128, KH, SC_G], F8, tag="xt")
                    nc.sync.dma_start(
                        out=xt,
                        in_=xT_d[:, SC_G * ch:SC_G * (ch + 1)]
                            .rearrange("(k p) s -> p k s", p=128))
                    if ch == 1:
                        # defer the remaining label loads + la matmuls until
                        # the projection pipeline is running
                        nc.sync.dma_start(
                            out=wla_sb,
                            in_=wlaT_d.rearrange("(k p) n -> p k n", p=128))
                        nc.sync.dma_start(
                            out=wp_sb,
                            in_=wpT_d.rearrange("(m p) n -> p m n", p=128))
                    if ch == 3:
                        for m in range(MC):
                            la_ps = ppc.tile([128, L], FP32, tag="ps",
                                             name="la_ps")
                            for k in range(KH):
                                nc.tensor.matmul(
                                    la_ps, wla_sb[:, k, 128 * m:128 * (m + 1)],
                                    lab_sb[:, k, :],
                                    start=(k == 0), stop=(k == KH - 1))
                            la_f = sg.tile([128, L], FP32, bufs=2,
                                           name="la_f", tag="la_f")
                            nc.scalar.activation(la_f, la_ps, AF.Sigmoid,
                                                 bias=bias_sb[:, m, 2:3])
                            # fold context in: laX = ctx[c] * sigmoid(...)
                            nc.vector.tensor_scalar_mul(laX_sb[:, m, :], la_f,
                                                        bias_sb[:, m, 3:4])

                    iaT = pacts.tile([128, MC, SC_G], BF, tag="iaT")
                    itN = pacts.tile([128, NSUB_G, C], BF, tag="itN")
                    ia_tiles.append(iaT)
                    it_tiles.append(itN)

                    # iaT[c, s] = sigmoid(Wia @ x.T + bia), c on partitions
                    for m in range(MC):
                        ia_ps = ppc.tile([128, SC_G], FP32, tag="ps", name="ia_ps")
                        for k in range(KH // 2):
                            nc.tensor.matmul(
                                ia_ps,
                                wcomb_sb[:, 2 * k:2 * k + 2,
                                         C + 128 * m:C + 128 * (m + 1)],
                                xt[:, 2 * k:2 * k + 2, :],
                                start=(k == 0), stop=(k == KH // 2 - 1),
                                perf_mode=mybir.MatmulPerfMode.DoubleRow)
                        nc.scalar.activation(iaT[:, m, :], ia_ps, AF.Sigmoid,
                                             bias=bias_sb[:, m, 0:1])

                    # it[s, c] = sigmoid(x @ Wi.T + bi), s on partitions
                    for j in range(NSUB_G):
                        it_ps = ppc.tile([128, SC_G], FP32, tag="ps", name="it_ps")
                        # bias via ones-row K=1 matmul (starts the group);
                        # skipped entirely when bi is known to be all-zero
                        skip_bias = zero_bi or False
                        if not skip_bias:
                            nc.tensor.matmul(it_ps, ones_sb, bi_sb,
                                             start=True, stop=False)
                        for k in range(KH):
                            nc.tensor.matmul(
                                it_ps,
                                xt[:, k, 128 * j:128 * (j + 1)],
                                wcomb_sb[:, k, 0:C],
                                start=(skip_bias and k == 0),
                                stop=(k == KH - 1))
                        nc.scalar.activation(itN[:, j, :], it_ps, AF.Sigmoid)

                # ---- phase 2: attention + fusion (all Exp after all Sigmoid).
                # Reuses the phase-1 PSUM pools (pp0 for logits, ppc for G and
                # the final output): no pool-boundary barrier between phases.
                with (tc.tile_pool(name="p2", bufs=6) as p2,
                      tc.tile_pool(name="p2s", bufs=12) as p2s):
                    USE_G = True
                    USE_ACC = True
                    for smp in range(B_LOC if True else 0):
                        G_ps = [ppc.tile([128, L], FP32, tag="ps", name=f"G{m}")
                                for m in range(MC)]
                        # all logits+softmax for the sample first, then all G
                        # matmuls: PE streams the logits groups back-to-back while
                        # the softmax (DVE/ACT) chains drain behind it, and the G
                        # stream then runs with every E ready -> no PE stalls
                        E_bs = []
                        for cc in range(CH_PER_SMP_G):
                            ch = smp * CH_PER_SMP_G + cc
                            iaT = ia_tiles[ch]
                            for j in range(NSUB_G):
                                lg_ps = pp0.tile([128, L], FP32, tag="lbl", name="lg_ps")
                                for m in range(MC):
                                    nc.tensor.matmul(
                                        lg_ps,
                                        iaT[:, m, 128 * j:128 * (j + 1)],
                                        laX_sb[:, m, :],
                                        start=(m == 0), stop=(m == MC - 1))
                                # softmax is shift-invariant; logits here are
                                # sums of 512 terms in [0,1] concentrated ~64+-4,
                                # so a fixed shift keeps exp() in fp32 range
                                # ([e-92, e+88] around the shift) with no
                                # per-row reduce_max on the DVE critical path.
                                E_f = p2.tile([128, L], FP32, tag="E_f")
                                den = p2s.tile([128, 1], FP32, tag="den")
                                if USE_ACC:
                                    nc.scalar.activation(E_f, lg_ps, AF.Exp,
                                                         bias=shift_sb,
                                                         accum_out=den)
                                else:
                                    nc.scalar.activation(E_f, lg_ps, AF.Exp,
                                                         bias=shift_sb)
                                    nc.vector.reduce_sum(den, E_f, axis=AX)
                                rr = p2s.tile([128, 1], FP32, tag="rr")
                                nc.vector.reciprocal(rr, den)
                                E_b = p2.tile([128, L], BF, tag="E_b", bufs=34)
                                nc.vector.tensor_scalar_mul(E_b, E_f, rr)
                                E_bs.append(E_b)
                        if USE_G:
                            # chunks 0..n-2: j-major (chases E production);
                            # last chunk: m-major so each G[m] closes early and
                            # its fusion epilogue overlaps the remaining G work
                            for cc in range(CH_PER_SMP_G - 1):
                                ch = smp * CH_PER_SMP_G + cc
                                itN = it_tiles[ch]
                                for j in range(NSUB_G):
                                    first = (cc == 0 and j == 0)
                                    for m in range(MC):
                                        nc.tensor.matmul(
                                            G_ps[m],
                                            itN[:, j, 128 * m:128 * (m + 1)],
                                            E_bs[cc * NSUB_G + j],
                                            start=first, stop=False,
                                            skip_group_check=True)
                            cc = CH_PER_SMP_G - 1
                            itN = it_tiles[smp * CH_PER_SMP_G + cc]
                            for m in range(MC):
                                for j in range(NSUB_G):
                                    nc.tensor.matmul(
                                        G_ps[m],
                                        itN[:, j, 128 * m:128 * (m + 1)],
                                        E_bs[cc * NSUB_G + j],
                                        start=False, stop=(j == NSUB_G - 1),
                                        skip_group_check=True)
                        # fusion[c] = sum_l G[c,l] * ltT[c,l]
                        if not USE_G:
                            nc.vector.memset(fus_f[:, 2 * smp:2 * smp + 1], 0.125)
                        else:
                            for m in range(MC):
                                gt = p2.tile([128, L], FP32, tag="gt")
                                nc.vector.tensor_mul(gt, G_ps[m], ltT_sb[:, m, :])
                                nc.vector.reduce_sum(
                                    fus_f[:, 2 * m + smp:2 * m + smp + 1],
                                    gt, axis=AX.X)

                    # final projection: out[b, h] = sum_c fus[c, b] * WpT[c, h]
                    if False:
                        nc.vector.memset(fus_f, 0.125)
                    nc.vector.tensor_copy(fus_b, fus_f)
                    for h2 in range(2):
                        o_ps = ppc.tile([B_LOC, 384], FP32, tag="ps", name="o_ps")
                        for m in range(MC):
                            nc.tensor.matmul(
                                o_ps,
                                fus_b[:, 2 * m:2 * (m + 1)],
                                wp_sb[:, m, 384 * h2:384 * (h2 + 1)],
                                start=(m == 0), stop=(m == MC - 1))
                        nc.scalar.copy(out_sb[:, 384 * h2:384 * (h2 + 1)], o_ps)
                    nc.sync.dma_start(out=out_d[:, :], in_=out_sb)

    nc.finalize()
    return nc


def _host_prep(inputs):
    """Pure layout prep: cast to bf16, transpose, concat. No FLOPs."""
    x = np.asarray(inputs["input_hidden_states"], np.float32)
    lab = np.asarray(inputs["label_hidden_states"], np.float32)
    Wi = np.asarray(inputs["Wi"], np.float32)
    Wia = np.asarray(inputs["Wia"], np.float32)
    Wl = np.asarray(inputs["Wl"], np.float32)
    Wla = np.asarray(inputs["Wla"], np.float32)
    Wp = np.asarray(inputs["Wp"], np.float32)

    # [H, B*S] transposed bf16 view of x, then per-core column shards
    x_bf = np.ascontiguousarray(x.reshape(B * S, H).T).astype(ml_dtypes.float8_e4m3)  # [H, B*S]

    wcombT = np.ascontiguousarray(
        np.concatenate([Wi, Wia], axis=0).T).astype(ml_dtypes.float8_e4m3)  # [H, 2C]
    wlT = np.ascontiguousarray(Wl.T).astype(BF16)                    # [H, C]
    wlaT = np.ascontiguousarray(Wla.T).astype(BF16)
    labT = np.ascontiguousarray(lab.T).astype(BF16)                  # [H, L]
    wpT = np.ascontiguousarray(Wp.T).astype(BF16)                    # [C, H]
    bi_row = np.asarray(inputs["bi"], np.float32).reshape(1, C).astype(BF16)
    bvec = np.stack([
        np.asarray(inputs["bia"], np.float32),
        np.asarray(inputs["bl"], np.float32),
        np.asarray(inputs["bla"], np.float32),
        np.asarray(inputs["context"], np.float32),
    ], axis=1)  # [C, 4]

    shared = dict(wcombT=wcombT, wlT=wlT, wlaT=wlaT, labT=labT, wpT=wpT,
                  bi_row=bi_row, bvec=bvec)
    in_maps = []
    for k in range(NCORES):
        m = dict(shared)
        m["xT"] = np.ascontiguousarray(x_bf[:, k * S_LOC:(k + 1) * S_LOC])
        in_maps.append(m)
    return in_maps




def _host_prep_general(inputs):
    """Layout prep for the general path (identical to the old baseline)."""
    x = np.asarray(inputs["input_hidden_states"], np.float32)
    Wi = np.asarray(inputs["Wi"], np.float32)
    Wia = np.asarray(inputs["Wia"], np.float32)
    Wl = np.asarray(inputs["Wl"], np.float32)
    Wla = np.asarray(inputs["Wla"], np.float32)
    Wp = np.asarray(inputs["Wp"], np.float32)
    lab = np.asarray(inputs["label_hidden_states"], np.float32)

    x_bf = np.ascontiguousarray(x.reshape(B * S, H).T).astype(ml_dtypes.float8_e4m3)
    wcombT = np.ascontiguousarray(
        np.concatenate([Wi, Wia], axis=0).T).astype(ml_dtypes.float8_e4m3)
    wlT = np.ascontiguousarray(Wl.T).astype(BF16)
    wlaT = np.ascontiguousarray(Wla.T).astype(BF16)
    labT = np.ascontiguousarray(lab.T).astype(BF16)
    wpT = np.ascontiguousarray(Wp.T).astype(BF16)
    bi_row = np.asarray(inputs["bi"], np.float32).reshape(1, C).astype(BF16)
    bvec = np.stack([
        np.asarray(inputs["bia"], np.float32),
        np.asarray(inputs["bl"], np.float32),
        np.asarray(inputs["bla"], np.float32),
        np.asarray(inputs["context"], np.float32),
    ], axis=1)

    shared = dict(wcombT=wcombT, wlT=wlT, wlaT=wlaT, labT=labT, wpT=wpT,
                  bi_row=bi_row, bvec=bvec)
    in_maps = []
    for k in range(NCORES):
        m = dict(shared)
        m["xT"] = np.ascontiguousarray(x_bf[:, k * S_LOC:(k + 1) * S_LOC])
        in_maps.append(m)
    return in_maps


LAST = {"exec_time_ns": None, "results": None}


def kernel(**inputs):
    zero_b = not any(
        np.any(np.asarray(inputs[k], np.float32))
        for k in ("bi", "bia", "bl", "bla"))
    key = "fast" if zero_b else "general"
    if key not in _cache:
        _cache[key] = build_fast() if zero_b else _build_general(zero_bi=False)
    nc = _cache[key]
    in_maps = host_prep_fast(inputs) if zero_b else _host_prep_general(inputs)
    res = None
    for attempt in range(3):
        try:
            res = run_bass_kernel_spmd(nc, in_maps,
                                       core_ids=list(range(NCORES)))
            break
        except Exception:
            # a previously-crashed session can leave the NeuronCores wedged;
            # the first execute fails and resets them, the retry succeeds
            if attempt == 2:
                raise
            time.sleep(3.0)
    LAST["exec_time_ns"] = res.exec_time_ns
    LAST["results"] = res
    if zero_b:
        return collect_fast(res.results)
    out = np.concatenate([res.results[k]["out"] for k in range(NCORES)], axis=0)
    return out.astype(np.float32)

